# revision 33
# baseline (speedup 1.0000x reference)
"""TRN2 Bass kernel for nn_CrossAttentionScorer.

The module collapses algebraically: seq-len is 1, so softmax over the single
attention score is identically 1.0 and the attention output equals `v`
exactly — the whole q/k path is dead code. The remaining computation is

    z   = layernorm(candidate)
    out = relu(z @ W.T + bh) @ sign_vec + b2

with W = (|w2| * (w1 @ wo @ wv) * ln_w) folded on the host and sign(w2)
handled by permuting FF columns so the final dot product becomes
sum(relu(pos block)) - sum(relu(neg block)).

All layernorm work happens on the host: z = rsig*(x-mu) is computed in
numpy, transposed, and cast to bf16 there, so the device does nothing but
bf16 matmuls (1 cycle/column at N=512, vs 1.5 for fp32r — and no PE
transposes at all) plus the relu/signed-reduce drain on VectorE. bf16
product noise is ~0.2% rel err, far inside the 2e-2 budget.

The matmul loop is ff-slice-major (nt outer, row-tile inner) so the
device can start computing after only ~1MB of DMA (half of the first
512-col weight slice + the first z chunk) instead of the full 4MB weight
burst; z stays SBUF-resident (8MB) and is reused by all four passes. All
input DMAs ride one HWDGE ring in exact consumption order — FIFO gives
the critical head data the full fabric bandwidth instead of sharing it
round-robin with the bulk streams. Per-pass partial sums land in
per-pass accumulator tiles; a short add/sub chain plus two 8KB output
DMAs (first half mid-pass, hiding the HBM write receipt) close the
kernel, so both the head and the tail of the timeline are near the
framework minimum. A burst of throwaway warmup matmuls during the
initial DMA wait flips the PE HAM clock gate to 2.4GHz before real work
arrives.

Data parallel over 8 NeuronCores: batch 32768 -> 8 x 4096 rows; weights
replicated. Per-core device work: [4096,1024] @ [1024,2048] bf16.
"""

import numpy as np

_B, _D, _FF = 32768, 1024, 2048
_NC = 8
_P = 128
_SHARD = _B // _NC     # 4096 rows per core
_NTILE = _SHARD // _P  # 32 tiles of 128 rows
_KC = _D // _P         # 8 contraction chunks
_NTW = 512             # matmul moving free size (one PSUM bank of fp32)
_NFT = _FF // _NTW     # 4 ff tiles
# z DMA chunk row counts: small head chunk (FIFO'd between the critical
# weight pieces on the sync ring) so the first matmul's semaphore fires
# early, then growing chunks with 1-2KB DMA lines
_ZCHUNKS = [256, 512, 512, 768, 1024, 1024]

_program_cache = {}


def _build_program(P0: int, has_bias: bool):
    import concourse.bacc as bacc
    import concourse.mybir as mybir
    import concourse.tile as tile
    from contextlib import ExitStack

    f32 = mybir.dt.float32
    bf16 = mybir.dt.bfloat16
    ALU = mybir.AluOpType
    AF = mybir.ActivationFunctionType

    nc = bacc.Bacc("TRN2", target_bir_lowering=False, debug=False)
    zt_d = nc.dram_tensor("zt", [_KC, _P, _SHARD], bf16, kind="ExternalInput")
    wt0_d = nc.dram_tensor("wt0", [_KC, _P, _NTW], bf16, kind="ExternalInput")
    wtr_d = nc.dram_tensor("wtr", [_KC, _P, _FF - _NTW], bf16,
                           kind="ExternalInput")
    if has_bias:
        bh_d = nc.dram_tensor("bh", [1, _FF], f32, kind="ExternalInput")
    o_d = nc.dram_tensor("o", [_P, _NTILE], f32, kind="ExternalOutput")

    # pos/neg split per ff tile (pos block is a prefix after host permutation)
    slices = {nt: [] for nt in range(_NFT)}
    for nt in range(_NFT):
        lo, hi = nt * _NTW, (nt + 1) * _NTW
        npos = min(max(P0 - lo, 0), _NTW)
        if npos > 0:
            slices[nt].append((0, npos, 1.0))
        if npos < _NTW:
            slices[nt].append((npos, _NTW, -1.0))
    pos_nts = [nt for nt in range(_NFT) if any(s[2] > 0 for s in slices[nt])]
    neg_nts = [nt for nt in range(_NFT) if any(s[2] < 0 for s in slices[nt])]

    with tile.TileContext(nc) as tc, ExitStack() as ctx:
        const = ctx.enter_context(tc.tile_pool(name="const", bufs=1))
        wpool = ctx.enter_context(tc.tile_pool(name="w", bufs=1))
        dpool = ctx.enter_context(tc.tile_pool(name="dump", bufs=1))
        spool = ctx.enter_context(tc.tile_pool(name="sp", bufs=2))
        psm = ctx.enter_context(tc.tile_pool(name="psm", bufs=8, space="PSUM"))

        # All input DMAs ride the sync HWDGE ring — FIFO per issuing
        # engine — in exact consumption order, so the critical first-tile
        # data gets the full fabric instead of round-robin-sharing it
        # with bulk streams: wt(nt0,kc0-3), z rows 0:256, wt(nt0,kc4-7),
        # z 256:768, then the remaining weight nt-slices interleaved with
        # progressively larger z chunks.
        wt_s = wpool.tile([_P, _KC, _FF], bf16)
        zfull = wpool.tile([_P, _KC, _SHARD], bf16)
        zt_r = zt_d.rearrange("kc p r -> p kc r")
        wt0_r = wt0_d.rearrange("kc p f -> p kc f")
        wtr_r = wtr_d.rearrange("kc p f -> p kc f")
        z_edges = [0]
        for n_ in _ZCHUNKS:
            z_edges.append(z_edges[-1] + n_)
        assert z_edges[-1] == _SHARD

        def load_z(c):
            nc.sync.dma_start(zfull[:, :, z_edges[c]:z_edges[c + 1]],
                              zt_r[:, :, z_edges[c]:z_edges[c + 1]])

        kh = _KC // 2
        nc.sync.dma_start(wt_s[:, 0:kh, 0:_NTW], wt0_r[:, 0:kh, :])
        load_z(0)
        nc.sync.dma_start(wt_s[:, kh:_KC, 0:_NTW], wt0_r[:, kh:_KC, :])
        for c in range(1, len(_ZCHUNKS)):
            load_z(c)
        # bulk weights last: pass nt needs its slice only ~55*nt us in
        for nt in range(1, _NFT):
            lo, hi = (nt - 1) * _NTW, nt * _NTW
            nc.sync.dma_start(wt_s[:, :, nt * _NTW:(nt + 1) * _NTW],
                              wtr_r[:, :, lo:hi])

        if has_bias:
            ones32 = const.tile([1, _P], f32)
            nc.gpsimd.memset(ones32[:], 1.0)
            ones = const.tile([1, _P], bf16)
            nc.vector.tensor_copy(ones[:], ones32[:])
            bh32 = const.tile([1, _FF], f32)
            bhr = const.tile([1, _FF], bf16)
            nc.sync.dma_start(bh32[:], bh_d[:, :])
            nc.vector.tensor_copy(bhr[:], bh32[:])

        # PE warmup: ~3.4us of throwaway matmuls during the initial DMA
        # wait flips the HAM clock gate to 2.4GHz before real work arrives
        warm = const.tile([_P, _NTW], bf16)
        nc.gpsimd.memset(warm[:], 0.0)
        # ~8 cold matmuls flip the HAM clock gate; the rest run warm and
        # keep the PE busy until the first real data lands (~12.5us), so
        # slow-DMA cores don't re-throttle and start their real matmuls
        # at the cold 1.2GHz clock
        wps = psm.tile([_P, _NTW], f32, name="ps", tag="ps")
        for i in range(20):
            nc.tensor.matmul(wps[:], warm[:, 0:_P], warm[:],
                             start=(i == 0), stop=(i == 19))

        # per-pass accumulators: pass nt adds its relu row-sums for tile t
        # into accP/accN[nt][:, t]
        accP = {nt: wpool.tile([_P, _NTILE], f32, name=f"accP{nt}")
                for nt in pos_nts}
        accN = {nt: wpool.tile([_P, _NTILE], f32, name=f"accN{nt}")
                for nt in neg_nts}
        ost = wpool.tile([_P, _NTILE], f32, name="ost")

        # combine: ost = sum(accP) - sum(accN), in two halves so the first
        # output DMA's completion receipt overlaps the last pass
        def combine_half(lo, hi):
            w = hi - lo

            def chain_sum(tiles, name):
                if len(tiles) == 1:
                    return tiles[0][:, lo:hi]
                s = spool.tile([_P, w], f32, tag=name, name=name)
                nc.vector.tensor_add(s[:], tiles[0][:, lo:hi],
                                     tiles[1][:, lo:hi])
                for t_ in tiles[2:]:
                    nc.vector.tensor_add(s[:], s[:], t_[:, lo:hi])
                return s

            pos_tiles = [accP[nt] for nt in pos_nts]
            neg_tiles = [accN[nt] for nt in neg_nts]
            if pos_tiles and neg_tiles:
                sP = chain_sum(pos_tiles, "sP")
                sN = chain_sum(neg_tiles, "sN")
                nc.vector.tensor_sub(ost[:, lo:hi], sP, sN)
            elif pos_tiles:
                sP = chain_sum(pos_tiles, "sP")
                nc.vector.tensor_copy(ost[:, lo:hi], sP)
            else:
                sN = chain_sum(neg_tiles, "sN")
                nc.vector.tensor_scalar_mul(ost[:, lo:hi], sN, -1.0)
            nc.sync.dma_start(o_d[:, lo:hi], ost[:, lo:hi])

        # tiles 0-1 of pass 0 split their kc loop into two accumulation
        # halves so the kc0-3 matmuls bridge the wait for the second
        # weight kc-half still in flight
        bridge = {}

        def mm_kc_range(ps, t, nt, k0, k1, final):
            for kc in range(k0, k1):
                nc.tensor.matmul(
                    ps[:],
                    zfull[:, kc, t * _P:(t + 1) * _P],
                    wt_s[:, kc, nt * _NTW:(nt + 1) * _NTW],
                    start=(kc == 0),
                    stop=(kc == _KC - 1 and final))

        for t in (0, 1):
            ps = psm.tile([_P, _NTW], f32, name="ps", tag="ps")
            mm_kc_range(ps, t, 0, 0, kh, not has_bias)
            bridge[t] = ps

        for nt in range(_NFT):
            for t in range(_NTILE):
                if nt == 0 and t in bridge:
                    ps = bridge.pop(t)
                    mm_kc_range(ps, t, 0, kh, _KC, not has_bias)
                else:
                    ps = psm.tile([_P, _NTW], f32, name="ps", tag="ps")
                    mm_kc_range(ps, t, nt, 0, _KC, not has_bias)
                if has_bias:
                    nc.tensor.matmul(ps[:], ones[:],
                                     bhr[:, nt * _NTW:(nt + 1) * _NTW],
                                     start=False, stop=True)
                # pos slices drain on the otherwise-idle ScalarE (relu +
                # row-sum via activation accum), neg slices on VectorE —
                # the two engines drain each psum tile in parallel
                hdump = dpool.tile([_P, _NTW], f32, tag="hdump")
                hdump2 = dpool.tile([_P, _NTW], f32, tag="hdump2")
                for (lo, hi, sgn) in slices[nt]:
                    if sgn > 0:
                        nc.scalar.activation(
                            hdump2[:, lo:hi], ps[:, lo:hi], AF.Relu,
                            accum_out=accP[nt][:, t:t + 1])
                    else:
                        nc.vector.tensor_scalar(
                            out=hdump[:, lo:hi], in0=ps[:, lo:hi],
                            scalar1=0.0, scalar2=None,
                            op0=ALU.max, op1=ALU.add,
                            accum_out=accN[nt][:, t:t + 1])
                if nt == _NFT - 1 and t in (_NTILE // 2 - 1, _NTILE - 1):
                    combine_half(t - _NTILE // 2 + 1, t + 1)

    nc.compile()
    return nc


def _get_program(P0: int, has_bias: bool):
    key = (P0, has_bias)
    if key not in _program_cache:
        _program_cache[key] = _build_program(P0, has_bias)
    return _program_cache[key]


def _fold_weights(inputs):
    gd = lambda k: np.asarray(inputs[k], dtype=np.float64)
    wv, wo, w1, w2 = gd("wv"), gd("wo"), gd("w1"), gd("w2")
    bv, bo, b1, b2 = gd("bv"), gd("bo"), gd("b1"), gd("b2")
    lnw, lnb = gd("ln_kv_w"), gd("ln_kv_b")

    M = w1 @ wo @ wv                              # [FF, D]
    bias_h = M @ lnb + w1 @ (wo @ bv + bo) + b1   # [FF]
    We = M * lnw[None, :]                         # fold LN weight into columns

    w2v = w2.reshape(-1)                          # [FF]
    aw2 = np.abs(w2v)
    sgn = np.sign(w2v)
    perm = np.argsort(-sgn, kind="stable")        # +1 block, then 0, then -1
    P0 = int((sgn >= 0).sum())

    Wf = (We * aw2[:, None])[perm]                # [FF, D]
    bf = (bias_h * aw2)[perm]                     # [FF]

    Wt = np.ascontiguousarray(Wf.T).astype(np.float32)   # [D, FF]
    bh = bf.astype(np.float32)[None, :]                  # [1, FF]
    has_bias = bool(np.any(bh != 0.0))
    return Wt, bh, has_bias, P0, float(b2.reshape(-1)[0])


def kernel(run_opts=None, **inputs):
    """Full inputs in, full [B, 1] float32 output out. 8-core data parallel."""
    import ml_dtypes
    from concourse.bass_utils import run_bass_kernel_spmd

    bf16 = ml_dtypes.bfloat16

    x = np.ascontiguousarray(np.asarray(inputs["candidate_feature"],
                                        dtype=np.float32))
    assert x.shape == (_B, _D)

    Wt, bh, has_bias, P0, b2 = _fold_weights(inputs)
    nc = _get_program(P0, has_bias)

    # host layernorm: z = rsig * (x - mu); lnw/lnb are folded into Wt/bh
    mu = x.mean(axis=1)
    s2 = np.einsum('bd,bd->b', x, x, optimize=True) / np.float32(_D)
    var = s2 - mu * mu
    rsig = 1.0 / np.sqrt(var + np.float32(1e-5))
    z = (x - mu[:, None]) * rsig[:, None]         # [B, D] f32

    wt_b = np.ascontiguousarray(Wt).astype(bf16).reshape(_KC, _P, _FF)
    wt0 = np.ascontiguousarray(wt_b[:, :, 0:_NTW])
    wtr = np.ascontiguousarray(wt_b[:, :, _NTW:_FF])

    common = {"wt0": wt0, "wtr": wtr}
    if has_bias:
        common["bh"] = bh
    in_maps = []
    for i in range(_NC):
        zt = np.ascontiguousarray(
            z[i * _SHARD:(i + 1) * _SHARD].T).astype(bf16)  # [D, SHARD]
        m = dict(common)
        m["zt"] = zt.reshape(_KC, _P, _SHARD)
        in_maps.append(m)

    res = run_bass_kernel_spmd(nc, in_maps, core_ids=list(range(_NC)),
                               **(run_opts or {}))
    # device output is [128, NTILE] per core with row = t*128 + p
    out = np.concatenate(
        [r["o"].T.reshape(_SHARD, 1) for r in res.results], axis=0)
    if b2 != 0.0:
        out = out + np.float32(b2)
    if run_opts:
        kernel.last_results = res
    return out.astype(np.float32)


# revision 34
# speedup vs baseline: 1.2056x; 1.2056x over previous
"""TRN2 Bass kernel for nn_CrossAttentionScorer.

The module collapses algebraically: seq-len is 1, so softmax over the single
attention score is identically 1.0 and the attention output equals `v`
exactly — the whole q/k path is dead code. The remaining computation is

    z   = layernorm(candidate)
    out = relu(z @ W.T + bh) @ sign_vec + b2

with W = (|w2| * (w1 @ wo @ wv) * ln_w) folded on the host and sign(w2)
handled by permuting FF columns so the final dot product becomes
sum(relu(pos block)) - sum(relu(neg block)).

All layernorm work happens on the host: z = rsig*(x-mu) is computed in
numpy, transposed, and cast to bf16 there, so the device does nothing but
bf16 matmuls (1 cycle/column at N=512, vs 1.5 for fp32r — and no PE
transposes at all) plus the relu/signed-reduce drain on VectorE. bf16
product noise is ~0.2% rel err, far inside the 2e-2 budget.

The matmul loop is ff-slice-major (nt outer, row-tile inner) so the
device can start computing after only ~1MB of DMA (half of the first
512-col weight slice + the first z chunk) instead of the full 4MB weight
burst; z stays SBUF-resident (8MB) and is reused by all four passes. All
input DMAs ride one HWDGE ring in exact consumption order — FIFO gives
the critical head data the full fabric bandwidth instead of sharing it
round-robin with the bulk streams. Per-pass partial sums land in
per-pass accumulator tiles; a short add/sub chain plus two 8KB output
DMAs (first half mid-pass, hiding the HBM write receipt) close the
kernel, so both the head and the tail of the timeline are near the
framework minimum. A burst of throwaway warmup matmuls during the
initial DMA wait flips the PE HAM clock gate to 2.4GHz before real work
arrives.

Data parallel over 8 NeuronCores: batch 32768 -> 8 x 4096 rows; weights
replicated. Per-core device work: [4096,1024] @ [1024,2048] bf16.
"""

import numpy as np

_B, _D, _FF = 32768, 1024, 2048
_NC = 8
_P = 128
_SHARD = _B // _NC     # 4096 rows per core
_NTILE = _SHARD // _P  # 32 tiles of 128 rows
_KC = _D // _P         # 8 contraction chunks
_NTW = 512             # matmul moving free size (one PSUM bank of fp32)
_NFT = _FF // _NTW     # 4 ff tiles
# z DMA chunk row counts: small head chunk (FIFO'd between the critical
# weight pieces on the sync ring) so the first matmul's semaphore fires
# early, then growing chunks with 1-2KB DMA lines
_ZCHUNKS = [256, 512, 512, 768, 1024, 1024]

_program_cache = {}


def _build_program(P0: int, has_bias: bool):
    import concourse.bacc as bacc
    import concourse.mybir as mybir
    import concourse.tile as tile
    from contextlib import ExitStack

    f32 = mybir.dt.float32
    bf16 = mybir.dt.bfloat16
    ALU = mybir.AluOpType
    AF = mybir.ActivationFunctionType

    nc = bacc.Bacc("TRN2", target_bir_lowering=False, debug=False)
    zt_d = nc.dram_tensor("zt", [_KC, _P, _SHARD], bf16, kind="ExternalInput")
    wt0_d = nc.dram_tensor("wt0", [_KC, _P, _NTW], bf16, kind="ExternalInput")
    wtr_d = nc.dram_tensor("wtr", [_KC, _P, _FF - _NTW], bf16,
                           kind="ExternalInput")
    if has_bias:
        bh_d = nc.dram_tensor("bh", [1, _FF], f32, kind="ExternalInput")
    o_d = nc.dram_tensor("o", [_P, _NTILE], f32, kind="ExternalOutput")

    # pos/neg split per ff tile (pos block is a prefix after host permutation)
    slices = {nt: [] for nt in range(_NFT)}
    for nt in range(_NFT):
        lo, hi = nt * _NTW, (nt + 1) * _NTW
        npos = min(max(P0 - lo, 0), _NTW)
        if npos > 0:
            slices[nt].append((0, npos, 1.0))
        if npos < _NTW:
            slices[nt].append((npos, _NTW, -1.0))
    pos_nts = [nt for nt in range(_NFT) if any(s[2] > 0 for s in slices[nt])]
    neg_nts = [nt for nt in range(_NFT) if any(s[2] < 0 for s in slices[nt])]

    with tile.TileContext(nc) as tc, ExitStack() as ctx:
        const = ctx.enter_context(tc.tile_pool(name="const", bufs=1))
        wpool = ctx.enter_context(tc.tile_pool(name="w", bufs=1))
        dpool = ctx.enter_context(tc.tile_pool(name="dump", bufs=1))
        spool = ctx.enter_context(tc.tile_pool(name="sp", bufs=2))
        psm = ctx.enter_context(tc.tile_pool(name="psm", bufs=8, space="PSUM"))

        # All input DMAs ride the sync HWDGE ring — FIFO per issuing
        # engine — in exact consumption order, so the critical first-tile
        # data gets the full fabric instead of round-robin-sharing it
        # with bulk streams: wt(nt0,kc0-3), z rows 0:256, wt(nt0,kc4-7),
        # z 256:768, then the remaining weight nt-slices interleaved with
        # progressively larger z chunks.
        wt_s = wpool.tile([_P, _KC, _FF], bf16)
        zfull = wpool.tile([_P, _KC, _SHARD], bf16)
        zt_r = zt_d.rearrange("kc p r -> p kc r")
        wt0_r = wt0_d.rearrange("kc p f -> p kc f")
        wtr_r = wtr_d.rearrange("kc p f -> p kc f")
        z_edges = [0]
        for n_ in _ZCHUNKS:
            z_edges.append(z_edges[-1] + n_)
        assert z_edges[-1] == _SHARD

        def load_z(c):
            nc.sync.dma_start(zfull[:, :, z_edges[c]:z_edges[c + 1]],
                              zt_r[:, :, z_edges[c]:z_edges[c + 1]])

        kh = _KC // 2
        nc.sync.dma_start(wt_s[:, 0:kh, 0:_NTW], wt0_r[:, 0:kh, :])
        load_z(0)
        nc.sync.dma_start(wt_s[:, kh:_KC, 0:_NTW], wt0_r[:, kh:_KC, :])
        for c in range(1, len(_ZCHUNKS)):
            load_z(c)
        # bulk weights last: pass nt needs its slice only ~55*nt us in
        for nt in range(1, _NFT):
            lo, hi = (nt - 1) * _NTW, nt * _NTW
            nc.sync.dma_start(wt_s[:, :, nt * _NTW:(nt + 1) * _NTW],
                              wtr_r[:, :, lo:hi])

        if has_bias:
            ones32 = const.tile([1, _P], f32)
            nc.gpsimd.memset(ones32[:], 1.0)
            ones = const.tile([1, _P], bf16)
            nc.vector.tensor_copy(ones[:], ones32[:])
            bh32 = const.tile([1, _FF], f32)
            bhr = const.tile([1, _FF], bf16)
            nc.sync.dma_start(bh32[:], bh_d[:, :])
            nc.vector.tensor_copy(bhr[:], bh32[:])

        # PE warmup: ~3.4us of throwaway matmuls during the initial DMA
        # wait flips the HAM clock gate to 2.4GHz before real work arrives
        warm = const.tile([_P, _NTW], bf16)
        nc.gpsimd.memset(warm[:], 0.0)
        # ~8 cold matmuls flip the HAM clock gate; the rest run warm and
        # keep the PE busy until the first real data lands (~12.5us), so
        # slow-DMA cores don't re-throttle and start their real matmuls
        # at the cold 1.2GHz clock
        wps = psm.tile([_P, _NTW], f32, name="ps", tag="ps")
        for i in range(20):
            nc.tensor.matmul(wps[:], warm[:, 0:_P], warm[:],
                             start=(i == 0), stop=(i == 19))

        # per-pass accumulators: pass nt adds its relu row-sums for tile t
        # into accP/accN[nt][:, t]
        accP = {nt: wpool.tile([_P, _NTILE], f32, name=f"accP{nt}")
                for nt in pos_nts}
        accN = {nt: wpool.tile([_P, _NTILE], f32, name=f"accN{nt}")
                for nt in neg_nts}
        ost = wpool.tile([_P, _NTILE], f32, name="ost")

        # combine: ost = sum(accP) - sum(accN), in two halves so the first
        # output DMA's completion receipt overlaps the last pass
        def combine_half(lo, hi):
            w = hi - lo

            def chain_sum(tiles, name):
                if len(tiles) == 1:
                    return tiles[0][:, lo:hi]
                s = spool.tile([_P, w], f32, tag=name, name=name)
                nc.vector.tensor_add(s[:], tiles[0][:, lo:hi],
                                     tiles[1][:, lo:hi])
                for t_ in tiles[2:]:
                    nc.vector.tensor_add(s[:], s[:], t_[:, lo:hi])
                return s

            pos_tiles = [accP[nt] for nt in pos_nts]
            neg_tiles = [accN[nt] for nt in neg_nts]
            if pos_tiles and neg_tiles:
                sP = chain_sum(pos_tiles, "sP")
                sN = chain_sum(neg_tiles, "sN")
                nc.vector.tensor_sub(ost[:, lo:hi], sP, sN)
            elif pos_tiles:
                sP = chain_sum(pos_tiles, "sP")
                nc.vector.tensor_copy(ost[:, lo:hi], sP)
            else:
                sN = chain_sum(neg_tiles, "sN")
                nc.vector.tensor_scalar_mul(ost[:, lo:hi], sN, -1.0)
            nc.sync.dma_start(o_d[:, lo:hi], ost[:, lo:hi])

        # tiles 0-1 of pass 0 split their kc loop into two accumulation
        # halves so the kc0-3 matmuls bridge the wait for the second
        # weight kc-half still in flight
        bridge = {}

        def mm_kc_range(ps, t, nt, k0, k1, final):
            for kc in range(k0, k1):
                nc.tensor.matmul(
                    ps[:],
                    zfull[:, kc, t * _P:(t + 1) * _P],
                    wt_s[:, kc, nt * _NTW:(nt + 1) * _NTW],
                    start=(kc == 0),
                    stop=(kc == _KC - 1 and final))

        for t in (0, 1):
            ps = psm.tile([_P, _NTW], f32, name="ps", tag="ps")
            mm_kc_range(ps, t, 0, 0, kh, not has_bias)
            bridge[t] = ps

        for nt in range(_NFT):
            for t in range(_NTILE):
                if nt == 0 and t in bridge:
                    ps = bridge.pop(t)
                    mm_kc_range(ps, t, 0, kh, _KC, not has_bias)
                else:
                    ps = psm.tile([_P, _NTW], f32, name="ps", tag="ps")
                    mm_kc_range(ps, t, nt, 0, _KC, not has_bias)
                if has_bias:
                    nc.tensor.matmul(ps[:], ones[:],
                                     bhr[:, nt * _NTW:(nt + 1) * _NTW],
                                     start=False, stop=True)
                hdump = dpool.tile([_P, _NTW], f32, tag="hdump")
                for (lo, hi, sgn) in slices[nt]:
                    tgt = accP[nt] if sgn > 0 else accN[nt]
                    nc.vector.tensor_scalar(
                        out=hdump[:, lo:hi], in0=ps[:, lo:hi],
                        scalar1=0.0, scalar2=None,
                        op0=ALU.max, op1=ALU.add,
                        accum_out=tgt[:, t:t + 1])
                if nt == _NFT - 1 and t in (_NTILE // 2 - 1, _NTILE - 1):
                    combine_half(t - _NTILE // 2 + 1, t + 1)

    nc.compile()
    return nc


def _get_program(P0: int, has_bias: bool):
    key = (P0, has_bias)
    if key not in _program_cache:
        _program_cache[key] = _build_program(P0, has_bias)
    return _program_cache[key]


def _fold_weights(inputs):
    gd = lambda k: np.asarray(inputs[k], dtype=np.float64)
    wv, wo, w1, w2 = gd("wv"), gd("wo"), gd("w1"), gd("w2")
    bv, bo, b1, b2 = gd("bv"), gd("bo"), gd("b1"), gd("b2")
    lnw, lnb = gd("ln_kv_w"), gd("ln_kv_b")

    M = w1 @ wo @ wv                              # [FF, D]
    bias_h = M @ lnb + w1 @ (wo @ bv + bo) + b1   # [FF]
    We = M * lnw[None, :]                         # fold LN weight into columns

    w2v = w2.reshape(-1)                          # [FF]
    aw2 = np.abs(w2v)
    sgn = np.sign(w2v)
    perm = np.argsort(-sgn, kind="stable")        # +1 block, then 0, then -1
    P0 = int((sgn >= 0).sum())

    Wf = (We * aw2[:, None])[perm]                # [FF, D]
    bf = (bias_h * aw2)[perm]                     # [FF]

    Wt = np.ascontiguousarray(Wf.T).astype(np.float32)   # [D, FF]
    bh = bf.astype(np.float32)[None, :]                  # [1, FF]
    has_bias = bool(np.any(bh != 0.0))
    return Wt, bh, has_bias, P0, float(b2.reshape(-1)[0])


def kernel(run_opts=None, **inputs):
    """Full inputs in, full [B, 1] float32 output out. 8-core data parallel."""
    import ml_dtypes
    from concourse.bass_utils import run_bass_kernel_spmd

    bf16 = ml_dtypes.bfloat16

    x = np.ascontiguousarray(np.asarray(inputs["candidate_feature"],
                                        dtype=np.float32))
    assert x.shape == (_B, _D)

    Wt, bh, has_bias, P0, b2 = _fold_weights(inputs)
    nc = _get_program(P0, has_bias)

    # host layernorm: z = rsig * (x - mu); lnw/lnb are folded into Wt/bh
    mu = x.mean(axis=1)
    s2 = np.einsum('bd,bd->b', x, x, optimize=True) / np.float32(_D)
    var = s2 - mu * mu
    rsig = 1.0 / np.sqrt(var + np.float32(1e-5))
    z = (x - mu[:, None]) * rsig[:, None]         # [B, D] f32

    wt_b = np.ascontiguousarray(Wt).astype(bf16).reshape(_KC, _P, _FF)
    wt0 = np.ascontiguousarray(wt_b[:, :, 0:_NTW])
    wtr = np.ascontiguousarray(wt_b[:, :, _NTW:_FF])

    common = {"wt0": wt0, "wtr": wtr}
    if has_bias:
        common["bh"] = bh
    in_maps = []
    for i in range(_NC):
        zt = np.ascontiguousarray(
            z[i * _SHARD:(i + 1) * _SHARD].T).astype(bf16)  # [D, SHARD]
        m = dict(common)
        m["zt"] = zt.reshape(_KC, _P, _SHARD)
        in_maps.append(m)

    res = run_bass_kernel_spmd(nc, in_maps, core_ids=list(range(_NC)),
                               **(run_opts or {}))
    # device output is [128, NTILE] per core with row = t*128 + p
    out = np.concatenate(
        [r["o"].T.reshape(_SHARD, 1) for r in res.results], axis=0)
    if b2 != 0.0:
        out = out + np.float32(b2)
    if run_opts:
        kernel.last_results = res
    return out.astype(np.float32)
